# revision 18
# baseline (speedup 1.0000x reference)
"""Trainium2 Bass kernel for nn_AttShare: dual-stream 1x1-conv attention.

Full-input contract: kernel(**inputs) takes the complete tensors from
setup_inputs() and returns (out1, out2) exactly like the reference.

Sharding (8 cores): 4 independent (batch, stream) attention units x 2-way
query-row split.  Each core gets the full x=[256,4096] of its unit (needed
for K and V over all N) plus its own 2048-column query slice, and produces
out[:, slice] = gamma * (V @ softmax(Q K^T)^T)[:, slice] + x[:, slice].

Key simplification: the reference adds a per-row bias (q . g) to the logits
before a row-softmax.  softmax is shift-invariant per row, so the entire
global-gating branch (pooled means -> MLP -> sigmoid -> bias) cancels and is
not computed.  The k-projection bias also shifts logits uniformly per row
and cancels; the q bias does not (it scales against k per column) and is
applied.  The v bias would add gamma*vb[c] (softmax rows sum to 1); it is
applied via a broadcast add on V^T tiles.

On-core dataflow (per core, fp32 I/O, matmuls in float32r = full PE rate):
  proj:  qq = Wq_dup @ xi (+qb)        [128, 2048]  (q/k duplicated on both
         kk = Wk_dup @ x  (+kb)        [128, 4096]   partition halves, which
         vt = gamma*(x^T @ Wv^T + vb)  [128 j, 32, 256]  enables QK packing)
         kk/vt interleaved per 512-column x chunk to follow DMA arrival.
  attn (2 phases of 1024 query columns, j streamed in row-packed pairs:
        chunk 2t on PE array rows 0-63, chunk 2t+1 on rows 64-127, running
        concurrently; software-pipelined one pair ahead):
         S^T tile = kk_slice^T @ qq    (K=64 contraction)      -> PSUM
         E = exp(S^T)  (ScalarE, PSUM->SBUF; no max-shift needed: |S|<~60,
                        and the softmax denominator normalizes later)
         Zacc += E     (VectorE)
         out_psum[c,i] += vt_j^T @ E   (PSUM-resident accumulation over j)
         Zbc = allones_128x128 @ Zacc  (colsum + partition-broadcast in one)
         out = out_psum * approx_recip(Zbc) + xi_slice  -> DRAM
  PSUM budget is exactly 8 banks: 4 output accumulators + 2x2-bank S^T tiles.
  Phase finalize is interleaved with the next phase's first QK pair to avoid
  a PE idle gap (> ~3.4us idle re-throttles the PE clock to 1.2GHz).

Measured (core 0, NTFF): ~160us when the chip is at full clock, ~195us when
power-throttled; rel. error vs fp32 reference ~1.1e-3 (float32r matmuls).
With BASS_ATT_MMDT=f32 (full-precision fp32 matmuls, 4 cycles/row): ~380us,
rel. error 2.6e-6.
"""

import os
import sys

import numpy as np

for _p in ("/opt/trn_rl_repo", os.path.expanduser("~/.axon_site/_ro/trn_rl_repo")):
    if os.path.isdir(_p) and _p not in sys.path:
        sys.path.insert(0, _p)

import concourse.bass as bass  # noqa: E402
import concourse.bacc as bacc  # noqa: E402
import concourse.mybir as mybir  # noqa: E402
import concourse.tile as tile  # noqa: E402

P = 128
C = 256         # channels
CQ = 64         # q/k channels
N = 4096        # H*W
NI = 2048       # query rows per core
PH = 1024       # query columns processed per phase
B, H, W = 2, 64, 64
F32 = mybir.dt.float32

# float32r: single-pass fp32 matmul (full PE rate at free-dim>=256, slightly
# reduced mantissa on HW).  Override with BASS_ATT_MMDT=f32 for full fp32.
_MMDT_NAME = os.environ.get("BASS_ATT_MMDT", "f32r")
MM_DT = mybir.dt.float32r if _MMDT_NAME == "f32r" else mybir.dt.float32
# BASS_ATT_SPLIT_QK=1: compute the attention logits with hi/lo-split q and k
# (3 fp32r matmuls ~= full fp32 accuracy at 0.75x the fp32 cost)
SPLIT_QK = os.environ.get("BASS_ATT_SPLIT_QK", "0") == "1"


def _r(ap):
    """View an AP in the matmul dtype (no-op: tiles are allocated in MM_DT)."""
    return ap


def _f(ap):
    """View an MM_DT AP as plain fp32 (for non-matmul engine reads)."""
    return ap.bitcast(F32) if MM_DT != F32 else ap


def _emit(tc, aps):
    nc = tc.nc
    import contextlib

    x_d, xi_d, wq_d, wk_d, wv_d, qb_d, kb_d, vb_d, gamma_d, out_d = aps
    EXP = mybir.ActivationFunctionType.Exp
    IDENT = mybir.ActivationFunctionType.Identity

    with contextlib.ExitStack() as ctx:
        singles = ctx.enter_context(tc.tile_pool(name="singles", bufs=1))
        pp = ctx.enter_context(tc.tile_pool(name="pp", bufs=4, space="PSUM"))
        p_s = ctx.enter_context(tc.tile_pool(name="p_s", bufs=2, space="PSUM"))
        etp = ctx.enter_context(tc.tile_pool(name="etp", bufs=8))
        zp = ctx.enter_context(tc.tile_pool(name="zp", bufs=2))
        outp = ctx.enter_context(tc.tile_pool(name="outp", bufs=4))

        # ---- loads, in consumption order -----------------------------------------
        # xi + wq + qb unlock the first projection; x streams in 4 column chunks.
        xi_sb = singles.tile([P, 2, NI], MM_DT)
        wq_sb = singles.tile([P, 2, P], MM_DT)
        wk_sb = singles.tile([P, 2, P], MM_DT)
        wv_sb = singles.tile([P, 2, C], MM_DT)
        x_sb = singles.tile([P, 2, N], MM_DT)
        xi_r = xi_d[:].rearrange("(o p) n -> p o n", p=P)
        x_r = x_d[:].rearrange("(o p) n -> p o n", p=P)

        def ld_xi(c):
            nc.sync.dma_start(out=xi_sb[:, :, bass.ts(c, NI // 4)],
                              in_=xi_r[:, :, bass.ts(c, NI // 4)])

        def ld_x(c):
            nc.sync.dma_start(out=x_sb[:, :, bass.ts(c, N // 8)],
                              in_=x_r[:, :, bass.ts(c, N // 8)])

        # phase 0 only consumes qq/xi columns 0..1023: load xi chunks 0-1 and
        # the weights first, then all of x, then the deferred xi half.
        ld_xi(0)
        ld_xi(1)
        nc.sync.dma_start(out=wq_sb, in_=wq_d[:].rearrange("(o p) m -> p o m", p=P))
        nc.sync.dma_start(out=wk_sb, in_=wk_d[:].rearrange("(o p) m -> p o m", p=P))
        nc.sync.dma_start(out=wv_sb, in_=wv_d[:].rearrange("(o p) m -> p o m", p=P))
        for c in range(8):
            ld_x(c)
        ld_xi(2)
        ld_xi(3)

        # small tensors via gpsimd SWDGE to keep the sync queue for bulk loads
        gamma_sb = singles.tile([1, 1], F32)
        nc.gpsimd.dma_start(out=gamma_sb, in_=gamma_d[:])
        qb_sb = singles.tile([P, 1], F32)
        nc.gpsimd.dma_start(out=qb_sb, in_=qb_d[:])
        kb_sb = singles.tile([P, 1], F32)
        nc.gpsimd.dma_start(out=kb_sb, in_=kb_d[:])
        vb_sb = singles.tile([1, C], F32)
        nc.gpsimd.dma_start(out=vb_sb, in_=vb_d[:])

        ones_jj = singles.tile([P, P], F32)  # all-ones lhsT: colsum + broadcast
        nc.vector.memset(ones_jj, 1.0)
        ones_1 = singles.tile([1, P], F32)   # lhsT for K=1 partition broadcast
        nc.vector.memset(ones_1, 1.0)
        ones_r = singles.tile([P, P], MM_DT)  # all-ones in matmul dtype (for et colsum)
        nc.vector.tensor_copy(ones_r, ones_jj)

        # broadcast gamma and gamma*vb across partitions via K=1 matmuls
        gamma_bc = singles.tile([P, 1], F32)
        pg = pp.tile([P, 1], F32, tag="pp", name="pg")
        nc.tensor.matmul(pg, ones_1, gamma_sb, start=True, stop=True)
        nc.vector.tensor_copy(gamma_bc, pg)
        gvb_bc = singles.tile([P, C], F32)
        pvb = pp.tile([P, C], F32, tag="pp")
        nc.tensor.matmul(pvb, ones_1, vb_sb, start=True, stop=True)
        nc.vector.tensor_scalar_mul(gvb_bc, pvb, gamma_bc)

        # ---- projections ---------------------------------------------------------
        qq_sb = singles.tile([P, NI], MM_DT)   # [q; q] duplicated across halves
        ql_sb = singles.tile([P, NI], MM_DT, name="ql_sb") if SPLIT_QK else None

        def qq_slice(s):
            ps = pp.tile([P, 512], F32, tag="pp", name=f"qq_ps_{s}")
            nc.tensor.matmul(ps, wq_sb[:, 0], xi_sb[:, 0, bass.ts(s, 512)],
                             start=True, stop=False)
            nc.tensor.matmul(ps, wq_sb[:, 1], xi_sb[:, 1, bass.ts(s, 512)],
                             start=False, stop=True)
            nc.scalar.activation(out=qq_sb[:, bass.ts(s, 512)], in_=ps, func=IDENT,
                                 bias=qb_sb, scale=1.0)
            if SPLIT_QK:  # lo remainder: (q_exact + qb) - round_r(q)
                nc.vector.scalar_tensor_tensor(
                    out=ql_sb[:, bass.ts(s, 512)], in0=ps, scalar=qb_sb,
                    in1=_f(qq_sb[:, bass.ts(s, 512)]),
                    op0=mybir.AluOpType.add, op1=mybir.AluOpType.subtract)

        qq_slice(0)
        qq_slice(1)

        kk_sb = singles.tile([P, N], MM_DT)    # [k; k] duplicated across halves
        kl_sb = singles.tile([P, N], MM_DT, name="kl_sb") if SPLIT_QK else None
        vt_sb = singles.tile([P, N // P, C], MM_DT)   # V^T: [j, c]

        def kk_slice(s, pool, tag):
            ps = pool.tile([P, 512], F32, tag=tag, name=f"kk_ps_{s}")
            nc.tensor.matmul(ps, wk_sb[:, 0], x_sb[:, 0, bass.ts(s, 512)],
                             start=True, stop=False)
            nc.tensor.matmul(ps, wk_sb[:, 1], x_sb[:, 1, bass.ts(s, 512)],
                             start=False, stop=True)
            nc.scalar.activation(out=kk_sb[:, bass.ts(s, 512)], in_=ps, func=IDENT,
                                 bias=kb_sb, scale=1.0)
            if SPLIT_QK:
                nc.vector.scalar_tensor_tensor(
                    out=kl_sb[:, bass.ts(s, 512)], in0=ps, scalar=kb_sb,
                    in1=_f(kk_sb[:, bass.ts(s, 512)]),
                    op0=mybir.AluOpType.add, op1=mybir.AluOpType.subtract)

        def vt_chunk(j, pool, tag):
            ps = pool.tile([P, C], F32, tag=tag, name=f"vt_ps_{j}")
            nc.tensor.matmul(ps, x_sb[:, 0, bass.ts(j, P)], wv_sb[:, 0],
                             start=True, stop=False)
            nc.tensor.matmul(ps, x_sb[:, 1, bass.ts(j, P)], wv_sb[:, 1],
                             start=False, stop=True)
            nc.vector.scalar_tensor_tensor(
                out=vt_sb[:, j], in0=ps, scalar=gamma_bc, in1=gvb_bc,
                op0=mybir.AluOpType.mult, op1=mybir.AluOpType.add)

        # consume x strictly in chunk-arrival order: kk slice s and vt chunks
        # 4s..4s+3 both read x columns [512s, 512s+512)
        for s in range(N // 512):
            kk_slice(s, pp, "pp")
            for j in range(4 * s, 4 * s + 4):
                vt_chunk(j, pp, "pp")
        qq_slice(2)   # phase-1 queries; xi chunks 2-3 arrive after x
        qq_slice(3)

        # ---- attention -----------------------------------------------------------
        # Row-packed QK: pair (jA, jB) = (2t, 2t+1); jA computed on array rows
        # 0-63 (operands on partitions 0:64), jB on rows 64-127 (partitions
        # 64:128, using the duplicated q/k halves) -> the two K=64 matmuls run
        # concurrently in the PE array.
        NPAIR = N // P // 2   # 16 pairs per phase
        NPH = NI // PH        # 2 phases

        def issue_pair(ph, t):
            # The two K=64 QK matmuls of a pair are emitted adjacently per
            # si-slice (array rows 0-63 / 64-127) so they overlap in the PE.
            i0 = ph * PH
            ab = []
            for h in range(2):
                ps = p_s.tile([P, PH], F32, tag="s", name=f"ps_{ph}_{t}_{h}")
                ab.append(ps)
            for si in range(PH // 512):
                for h, j in ((0, 2 * t), (1, 2 * t + 1)):
                    lo = h * CQ
                    out = ab[h][:, bass.ts(si, 512)]
                    kkj = kk_sb[lo:lo + CQ, bass.ts(j, P)]
                    qqi = qq_sb[lo:lo + CQ, bass.ds(i0 + si * 512, 512)]
                    if not SPLIT_QK:
                        nc.tensor.matmul(out, kkj, qqi, start=True, stop=True)
                    else:
                        klj = kl_sb[lo:lo + CQ, bass.ts(j, P)]
                        qli = ql_sb[lo:lo + CQ, bass.ds(i0 + si * 512, 512)]
                        nc.tensor.matmul(out, kkj, qqi, start=True, stop=False)
                        nc.tensor.matmul(out, kkj, qli, start=False, stop=False)
                        nc.tensor.matmul(out, klj, qqi, start=False, stop=True)
            ets = []
            for h in range(2):
                et = etp.tile([P, PH], MM_DT, tag="et", name=f"et_{ph}_{t}_{h}")
                nc.scalar.activation(out=et, in_=ab[h], func=EXP, scale=1.0)
                ets.append(et)
            return ets

        state = {}

        def pv_half(po, t, h, et):
            j = 2 * t + h
            for cc in range(C // P):
                for si in range(PH // 512):
                    nc.tensor.matmul(
                        po[cc][si],
                        vt_sb[:, j, bass.ts(cc, P)],
                        et[:, bass.ts(si, 512)],
                        start=(t == 0 and h == 0), stop=(t == NPAIR - 1 and h == 1))

        def finalize(st, etA, etB):
            ph, zacc, po = st[0], st[1], st[2]
            i0 = ph * PH
            # Z broadcast to all partitions in one shot: all-ones lhsT colsum.
            # zacc covers pairs 0..NPAIR-2; the last pair's exp tiles are
            # summed directly by the PE (avoids waiting for the DVE chain).
            pzb = p_s.tile([P, PH], F32, tag="s", name=f"pzb_{ph}")
            for si in range(PH // 512):
                sl = bass.ts(si, 512)
                nc.tensor.matmul(pzb[:, sl], ones_jj, zacc[:, sl],
                                 start=True, stop=False)
                nc.tensor.matmul(pzb[:, sl], ones_r, etA[:, sl],
                                 start=False, stop=False)
                nc.tensor.matmul(pzb[:, sl], ones_r, etB[:, sl],
                                 start=False, stop=True)
            zbc = zp.tile([P, PH], F32, tag="zbc", name=f"zbc_{ph}")
            scr = zp.tile([P, PH], F32, tag="scr", name=f"scr_{ph}")
            nc.vector.reciprocal_approx_accurate(out=zbc, in_=pzb, scratch=scr)
            for cc in range(C // P):
                ob = outp.tile([P, PH], F32, tag="ob", name=f"ob_{ph}_{cc}")
                for si in range(PH // 512):
                    nc.vector.tensor_mul(ob[:, bass.ts(si, 512)], po[cc][si],
                                         zbc[:, bass.ts(si, 512)])
                for si in range(PH // 512):
                    sl = bass.ds(i0 + si * 512, 512)
                    nc.vector.tensor_add(ob[:, bass.ts(si, 512)],
                                         ob[:, bass.ts(si, 512)], _f(xi_sb[:, cc, sl]))
                    nc.sync.dma_start(
                        out=out_d[:].rearrange("(o p) n -> p o n", p=P)[:, cc, sl],
                        in_=ob[:, bass.ts(si, 512)])

        pend = {(0, 0): issue_pair(0, 0)}
        for ph in range(NPH):
            zacc = zp.tile([P, PH], F32, tag="z", name=f"zacc_{ph}")
            po = [[pp.tile([P, 512], F32, tag="pp", name=f"po_{ph}_{cc}_{si}")
                   for si in range(PH // 512)]
                  for cc in range(C // P)]
            state[ph] = (ph, zacc, po)
            for t in range(NPAIR):
                etA, etB = pend.pop((ph, t))
                pv_half(po, t, 0, etA)
                nxt = (ph, t + 1) if t + 1 < NPAIR else (
                    (ph + 1, 0) if ph + 1 < NPH else None)
                if nxt is not None:
                    pend[nxt] = issue_pair(*nxt)
                pv_half(po, t, 1, etB)
                if t == NPAIR - 1:
                    state[ph] = state[ph] + (etA, etB)
                elif t == 0:
                    nc.vector.tensor_copy(zacc, _f(etA))
                    nc.vector.tensor_add(zacc, zacc, _f(etB))
                else:
                    nc.vector.tensor_add(zacc, zacc, _f(etA))
                    nc.vector.tensor_add(zacc, zacc, _f(etB))
                if ph > 0 and t == 0:
                    st = state.pop(ph - 1)
                    finalize(st[:3], st[3], st[4])
        st = state.pop(NPH - 1)
        finalize(st[:3], st[3], st[4])


def _build_nc():
    nc = bacc.Bacc(trn_type="TRN2", target_bir_lowering=False, debug=False)
    aps = (
        nc.declare_dram_parameter("x", [C, N], MM_DT, isOutput=False),
        nc.declare_dram_parameter("xi", [C, NI], MM_DT, isOutput=False),
        nc.declare_dram_parameter("wqT", [C, P], MM_DT, isOutput=False),
        nc.declare_dram_parameter("wkT", [C, P], MM_DT, isOutput=False),
        nc.declare_dram_parameter("wvT", [C, C], MM_DT, isOutput=False),
        nc.declare_dram_parameter("qb", [P, 1], F32, isOutput=False),
        nc.declare_dram_parameter("kb", [P, 1], F32, isOutput=False),
        nc.declare_dram_parameter("vb", [1, C], F32, isOutput=False),
        nc.declare_dram_parameter("gamma", [1, 1], F32, isOutput=False),
        nc.declare_dram_parameter("out", [C, NI], F32, isOutput=True),
    )
    with tile.TileContext(nc) as tc:
        _emit(tc, aps)
    nc.compile()
    return nc


_NC_CACHE = {}


def get_nc():
    if "nc" not in _NC_CACHE:
        _NC_CACHE["nc"] = _build_nc()
    return _NC_CACHE["nc"]


def make_in_maps(inputs):
    """Build the 8 per-core input maps from the full problem inputs."""
    f = np.float32
    x_streams = [
        np.ascontiguousarray(inputs["input1"].reshape(B, C, N), dtype=f),
        np.ascontiguousarray(inputs["input2"].reshape(B, C, N), dtype=f),
    ]
    wsets = []
    for s in ("1", "2"):
        qw = np.asarray(inputs[f"q{s}_w"], dtype=f)
        kw = np.asarray(inputs[f"k{s}_w"], dtype=f)
        vw = np.asarray(inputs[f"v{s}_w"], dtype=f)
        qb = np.asarray(inputs[f"q{s}_b"], dtype=f)
        kb = np.asarray(inputs[f"k{s}_b"], dtype=f)
        vb = np.asarray(inputs[f"v{s}_b"], dtype=f)
        wsets.append(dict(
            wqT=np.ascontiguousarray(np.concatenate([qw, qw], 0).T),
            wkT=np.ascontiguousarray(np.concatenate([kw, kw], 0).T),
            wvT=np.ascontiguousarray(vw.T),
            qb=np.ascontiguousarray(np.concatenate([qb, qb])[:, None]),
            kb=np.ascontiguousarray(np.concatenate([kb, kb])[:, None]),
            vb=np.ascontiguousarray(vb[None, :]),
        ))
    gamma = np.ascontiguousarray(np.asarray(inputs["gamma"], dtype=f).reshape(1, 1))

    in_maps = []
    for core in range(8):
        u, h = core // 2, core % 2
        b, s = u // 2, u % 2
        xs = x_streams[s][b]
        m = dict(wsets[s])
        m["x"] = xs
        m["xi"] = np.ascontiguousarray(xs[:, h * NI:(h + 1) * NI])
        m["gamma"] = gamma
        in_maps.append(m)
    return in_maps


def assemble(results, inputs):
    """Stitch the 8 per-core [256, 2048] outputs into (out1, out2)."""
    outs = [np.empty((B, C, N), np.float32) for _ in range(2)]
    for core in range(8):
        u, h = core // 2, core % 2
        b, s = u // 2, u % 2
        outs[s][b][:, h * NI:(h + 1) * NI] = results[core]["out"]
    out1 = outs[0].reshape(B, C, H, W)
    out2 = outs[1].reshape(B, C, H, W)
    return out1, out2


def kernel(**inputs):
    from concourse.bass_utils import run_bass_kernel_spmd

    nc = get_nc()
    in_maps = make_in_maps(inputs)
    res = run_bass_kernel_spmd(nc, in_maps, list(range(8)))
    return assemble(res.results, inputs)

